# revision 6
# baseline (speedup 1.0000x reference)
"""DBSCAN fragmenter (connected components of eps-neighborhood graph) on 8 Trainium2 cores.

Key structural fact: adjacency requires equal batch id AND equal semantic
class, so the graph splits into 16 independent (bid,sem) groups (~512 points
each). Host-side we stably sort points by group and give each core 2 whole
groups; all propagation is then core-local -- no collectives at all.

Per core (single SPMD program, uniform shapes):
  - rows: 2 groups x R tiles of 128 (pad rows are far-away points)
  - cols: 2 groups x C columns (C = max real group size, pad cols far away)
  - D[i,j] = clamp(S*(d2(i,j) - 3), 0, 24576) as int16 via one K=12 bf16
    matmul per tile (exact: coords<=255 and q split into 8-bit digits; all
    values are small-int times power of two) + one clamped store.
  - adjacency (d2<=3, integer d2, eps=1.999) <=> D=0; else D>=8192 > labels.
  - 2 rounds of min-label propagation (component ecc from root <= 2):
    per tile: M = max(D, labels_bcast) [DVE TT, 2x i16 mode], then
    labels_new = free-axis min via tensor_scalar+accum_out [4x mode].
    Label broadcast between rounds via DRAM round-trip + broadcast DMA.
  - counts: per tile tensor_scalar(is_equal)+accum_out(add); out = count>=3 ?
    label : -1.
Labels are core-local column indices; the host maps roots back to original
point indices (stable sort keeps within-group order = original index order).
"""
import sys
sys.path.insert(0, "/opt/trn_rl_repo")
import numpy as np

NCORES = 8
NGROUPS = 16
W = 64.0          # batch/class separation weight (64*2=128 > 2*eps guard; (64*1)^2=4096 > 3)
S = 8192.0        # distance scale: S*1 > max local label (< 2*C ~ 1300)
PADB = 320.0      # pad-point batch coordinate (W*5): >= (320-192)^2 from all real
CLAMP = 24576.0   # D clamp: > 8191 >= any label, and 1231 + 24576 < 32767
STORE_MODE = "clamp"   # "clamp": DVE/Pool clamped stores (interp-exact)
                       # "act":   ACT relu stores (relies on HW i16 saturation)

_CACHE = {}


def _build(R, C):
    import concourse.bass as bass
    import concourse.bacc as bacc
    import concourse.mybir as mybir
    import concourse.tile as tile

    f32 = mybir.dt.float32
    bf16 = mybir.dt.bfloat16
    i16 = mybir.dt.int16
    i32 = mybir.dt.int32
    OP = mybir.AluOpType
    AF = mybir.ActivationFunctionType

    T = 2 * R               # row tiles per core
    COLS = 2 * C            # columns per core
    NROWS = T * 128

    nc = bacc.Bacc("TRN2", target_bir_lowering=False, debug=False,
                   num_devices=NCORES)

    Wt_in = nc.dram_tensor("Wt", [12, NROWS], bf16, kind="ExternalInput")
    Xt_in = nc.dram_tensor("Xt", [12, COLS], bf16, kind="ExternalInput")
    iota_in = nc.dram_tensor("iota", [1, COLS], i16, kind="ExternalInput")
    out_t = nc.dram_tensor("out", [1, NROWS], i32, kind="ExternalOutput")

    with tile.TileContext(nc) as tc:
        with (
            tc.tile_pool(name="po", bufs=1) as po,
            tc.tile_pool(name="ps", bufs=4, space="PSUM") as pp,
            tc.tile_pool(name="dram", bufs=1, space="DRAM") as dram,
        ):
            iotaB = po.tile([128, COLS], i16, tag="iotaB")
            nc.sync.dma_start(iotaB[:], iota_in[0:1, :].to_broadcast((128, COLS)))
            Wt = po.tile([12, NROWS], bf16, tag="Wt")
            nc.sync.dma_start(Wt[:], Wt_in[:])
            Xt = po.tile([12, COLS], bf16, tag="Xt")
            nc.sync.dma_start(Xt[:], Xt_in[:])

            D = po.tile([128, T * C], i16, tag="D")
            M = [po.tile([128, C], i16, tag=f"M{k}", name=f"M{k}") for k in range(2)]
            M2 = [po.tile([128, C], i16, tag=f"M2{k}", name=f"M2{k}") for k in range(2)]
            Mb = [po.tile([128, C], bf16, tag=f"Mb{k}", name=f"Mb{k}") for k in range(2)]
            l1colf = po.tile([128, T], f32, tag="l1colf")
            l2colf = po.tile([128, T], f32, tag="l2colf")
            l1col = po.tile([128, T], i16, tag="l1col")
            l2col = po.tile([128, T], i16, tag="l2col")
            labelB = po.tile([128, COLS], i16, tag="labelB")
            labelB2 = po.tile([128, COLS], i16, tag="labelB2")
            cnt = po.tile([128, T], f32, tag="cnt")

            l1row = dram.tile([1, NROWS], i16, tag="l1row", name="l1row")
            l2row = dram.tile([1, NROWS], i16, tag="l2row", name="l2row")

            def store(dst, ps, t):
                # GPSIMD cannot read PSUM, so stores are DVE (clamp) or ACT.
                if STORE_MODE == "act":
                    nc.scalar.activation(dst, ps[:], AF.Relu, bias=0.0, scale=1.0)
                else:
                    nc.vector.tensor_scalar(out=dst, in0=ps[:], scalar1=0.0,
                                            scalar2=CLAMP, op0=OP.max, op1=OP.min)

            def tt_engine(t):
                return nc.vector

            def labels_to_bcast(colf, col, row_d, dstB, grp):
                # f32 accum -> i16, then [128, R] -> DRAM row -> bcast tile
                nc.vector.tensor_copy(col[:, grp * R:(grp + 1) * R],
                                      colf[:, grp * R:(grp + 1) * R])
                nc.sync.dma_start(
                    row_d[0:1, grp * R * 128:(grp + 1) * R * 128]
                    .rearrange("o (t p) -> (o p) t", p=128),
                    col[:, grp * R:(grp + 1) * R])
                nc.sync.dma_start(
                    dstB[:, grp * C:(grp + 1) * C],
                    row_d[0:1, grp * R * 128:grp * R * 128 + C]
                    .to_broadcast((128, C)))

            # ---- build D + iteration 1 (tile-pipelined) ----
            for t in range(T):
                grp = t // R
                ps = pp.tile([128, C], f32, tag="ps")
                w = Wt[:, t * 128:(t + 1) * 128]
                nc.tensor.matmul(ps[:, 0:512], w, Xt[:, grp * C:grp * C + 512])
                if C > 512:
                    nc.tensor.matmul(ps[:, 512:C], w,
                                     Xt[:, grp * C + 512:grp * C + C])
                dst = D[:, t * C:(t + 1) * C]
                store(dst, ps, t)
                tt_engine(t).tensor_tensor(M[t % 2][:], dst,
                                           iotaB[:, grp * C:(grp + 1) * C], OP.max)
                nc.vector.tensor_scalar(out=M2[t % 2][:], in0=M[t % 2][:],
                                        scalar1=0.0, scalar2=None,
                                        op0=OP.add, op1=OP.min,
                                        accum_out=l1colf[:, t:t + 1])
                if t == R - 1:
                    labels_to_bcast(l1colf, l1col, l1row, labelB, 0)
                elif t == T - 1:
                    labels_to_bcast(l1colf, l1col, l1row, labelB, 1)

            # ---- iteration 2 ----
            for t in range(T):
                grp = t // R
                dst = D[:, t * C:(t + 1) * C]
                tt_engine(t).tensor_tensor(M[t % 2][:], dst,
                                           labelB[:, grp * C:(grp + 1) * C], OP.max)
                nc.vector.tensor_scalar(out=M2[t % 2][:], in0=M[t % 2][:],
                                        scalar1=0.0, scalar2=None,
                                        op0=OP.add, op1=OP.min,
                                        accum_out=l2colf[:, t:t + 1])
                if t == R - 1:
                    labels_to_bcast(l2colf, l2col, l2row, labelB2, 0)
                elif t == T - 1:
                    labels_to_bcast(l2colf, l2col, l2row, labelB2, 1)

            # ---- counts + min-size filter ----
            for t in range(T):
                grp = t // R
                nc.vector.tensor_scalar(out=Mb[t % 2][:],
                                        in0=labelB2[:, grp * C:(grp + 1) * C],
                                        scalar1=l2colf[:, t:t + 1], scalar2=None,
                                        op0=OP.is_equal, op1=OP.add,
                                        accum_out=cnt[:, t:t + 1])
            mask = po.tile([128, T], f32, tag="mask")
            nc.vector.tensor_scalar(out=mask[:], in0=cnt[:], scalar1=2.5,
                                    scalar2=None, op0=OP.is_ge)
            lp1 = po.tile([128, T], f32, tag="lp1")
            nc.vector.tensor_scalar(out=lp1[:], in0=l2colf[:], scalar1=1.0,
                                    scalar2=None, op0=OP.add)
            sel = po.tile([128, T], f32, tag="sel")
            nc.vector.tensor_tensor(sel[:], mask[:], lp1[:], OP.mult)
            outf = po.tile([128, T], f32, tag="outf")
            nc.vector.tensor_scalar(out=outf[:], in0=sel[:], scalar1=-1.0,
                                    scalar2=None, op0=OP.add)
            outi = po.tile([128, T], i32, tag="outi")
            nc.vector.tensor_copy(outi[:], outf[:])
            nc.sync.dma_start(
                out_t[0:1, :].rearrange("o (t p) -> (o p) t", p=128), outi[:])

    nc.compile()
    return nc


def _layout(data):
    """Host-side: stable group sort, padding, bf16 operand prep."""
    import ml_dtypes
    data = np.asarray(data, np.float32)
    N = data.shape[0]
    bid = data[:, 0].astype(np.int64)
    sem = data[:, 4].astype(np.int64)
    xyz = data[:, 1:4].astype(np.int64)
    g = bid * 4 + sem
    order = np.argsort(g, kind="stable")
    sizes = np.bincount(g, minlength=NGROUPS)
    starts = np.concatenate([[0], np.cumsum(sizes)])
    gidx = [order[starts[k]:starts[k + 1]] for k in range(NGROUPS)]

    C = int(sizes.max())
    R = (C + 127) // 128
    T = 2 * R

    # 5D feature rows per point: [x, y, z, W*b, W*s]; pads: [0,0,0,PADB,0]
    def feats(idx, n_slots):
        f = np.zeros((5, n_slots), np.int64)
        k = len(idx)
        f[0:3, :k] = xyz[idx].T
        f[3, :k] = (W * bid[idx]).astype(np.int64)
        f[4, :k] = (W * sem[idx]).astype(np.int64)
        f[3, k:] = int(PADB)
        return f

    in_maps = []
    meta = []
    for c in range(NCORES):
        Wt = np.zeros((12, T * 128), np.float64)
        Xt = np.zeros((12, 2 * C), np.float64)
        for grp in range(2):
            idx = gidx[2 * c + grp]
            fr = feats(idx, R * 128)   # row slots
            fc = feats(idx, C)         # col slots
            qr = (fr * fr).sum(axis=0)
            qc = (fc * fc).sum(axis=0)
            rs, cs = grp * R * 128, grp * C
            Wt[0:5, rs:rs + R * 128] = fr
            Wt[5, rs:rs + R * 128] = qr >> 16
            Wt[6, rs:rs + R * 128] = (qr >> 8) & 255
            Wt[7, rs:rs + R * 128] = qr & 255
            Wt[8:12, rs:rs + R * 128] = 1.0
            Xt[0:5, cs:cs + C] = -2.0 * S * fc
            Xt[5, cs:cs + C] = S * 65536.0
            Xt[6, cs:cs + C] = S * 256.0
            Xt[7, cs:cs + C] = S
            Xt[8, cs:cs + C] = S * 65536.0 * (qc >> 16)
            Xt[9, cs:cs + C] = S * 256.0 * ((qc >> 8) & 255)
            Xt[10, cs:cs + C] = S * (qc & 255)
            Xt[11, cs:cs + C] = -3.0 * S
        Wt_b = Wt.astype(np.float32).astype(ml_dtypes.bfloat16)
        Xt_b = Xt.astype(np.float32).astype(ml_dtypes.bfloat16)
        assert np.array_equal(Wt_b.astype(np.float64), Wt), "Wt not bf16-exact"
        assert np.array_equal(Xt_b.astype(np.float64), Xt), "Xt not bf16-exact"
        iota = np.arange(2 * C, dtype=np.int16).reshape(1, -1)
        in_maps.append({"Wt": Wt_b, "Xt": Xt_b, "iota": iota})
        meta.append((gidx[2 * c], gidx[2 * c + 1]))
    return in_maps, meta, R, C, N


def kernel(data: np.ndarray) -> np.ndarray:
    from concourse.bass_utils import run_bass_kernel_spmd

    in_maps, meta, R, C, N = _layout(data)
    key = ("nc", R, C)
    if key not in _CACHE:
        _CACHE[key] = _build(R, C)
        _CACHE["nc"] = _CACHE[key]
    nc = _CACHE[key]
    res = run_bass_kernel_spmd(nc, in_maps, core_ids=list(range(NCORES)))

    out = np.full(N, -1, np.int32)
    for c in range(NCORES):
        o = np.asarray(res.results[c]["out"]).reshape(-1)
        for grp in range(2):
            idx = meta[c][grp]
            sz = len(idx)
            vals = o[grp * R * 128: grp * R * 128 + sz]
            ok = (vals >= grp * C) & (vals < grp * C + sz)
            out[idx[ok]] = idx[vals[ok] - grp * C]
            out[idx[~ok & (vals >= 0)]] = -2   # unexpected: root outside group
    return out


# revision 8
# speedup vs baseline: 1.1993x; 1.1993x over previous
"""DBSCAN fragmenter (connected components of eps-neighborhood graph) on 8 Trainium2 cores.

Key structural fact: adjacency requires equal batch id AND equal semantic
class, so the graph splits into 16 independent (bid,sem) groups (~512 points
each). Host-side we stably sort points by group and give each core 2 whole
groups; all propagation is then core-local -- no collectives at all.

Per core (single SPMD program, uniform shapes):
  - rows: 2 groups x R tiles of 128 (pad rows are far-away points)
  - cols: 2 groups x C columns (C = max real group size, pad cols far away)
  - D[i,j] = clamp(S*(d2(i,j) - 3), 0, 24576) as int16 via one K=12 bf16
    matmul per tile (exact: coords<=255 and q split into 8-bit digits; all
    values are small-int times power of two) + one clamped store.
  - adjacency (d2<=3, integer d2, eps=1.999) <=> D=0; else D>=8192 > labels.
  - 2 rounds of min-label propagation (component ecc from root <= 2):
    per tile: M = max(D, labels_bcast) [DVE TT, 2x i16 mode], then
    labels_new = free-axis min via tensor_scalar+accum_out [4x mode].
    Label broadcast between rounds via DRAM round-trip + broadcast DMA.
  - counts: per tile tensor_scalar(is_equal)+accum_out(add); out = count>=3 ?
    label : -1.
Labels are core-local column indices; the host maps roots back to original
point indices (stable sort keeps within-group order = original index order).
"""
import sys
sys.path.insert(0, "/opt/trn_rl_repo")
import numpy as np

NCORES = 8
NGROUPS = 16
W = 64.0          # batch/class separation weight (64*2=128 > 2*eps guard; (64*1)^2=4096 > 3)
S = 8192.0        # distance scale: S*1 > max local label (< 2*C ~ 1300)
PADB = 320.0      # pad-point batch coordinate (W*5): >= (320-192)^2 from all real
CLAMP = 24576.0   # D clamp: > 8191 >= any label, and 1231 + 24576 < 32767
STORE_MODE = "act"   # "clamp": DVE/Pool clamped stores (interp-exact)
                       # "act":   ACT relu stores (relies on HW i16 saturation)

_CACHE = {}


def _build(R, C):
    import concourse.bass as bass
    import concourse.bacc as bacc
    import concourse.mybir as mybir
    import concourse.tile as tile

    f32 = mybir.dt.float32
    bf16 = mybir.dt.bfloat16
    i16 = mybir.dt.int16
    i32 = mybir.dt.int32
    OP = mybir.AluOpType
    AF = mybir.ActivationFunctionType

    T = 2 * R               # row tiles per core
    COLS = 2 * C            # columns per core
    NROWS = T * 128

    nc = bacc.Bacc("TRN2", target_bir_lowering=False, debug=False,
                   num_devices=NCORES)

    Wt_in = nc.dram_tensor("Wt", [12, NROWS], bf16, kind="ExternalInput")
    Xt_in = nc.dram_tensor("Xt", [12, COLS], bf16, kind="ExternalInput")
    iota_in = nc.dram_tensor("iota", [1, COLS], i16, kind="ExternalInput")
    out_t = nc.dram_tensor("out", [1, NROWS], i32, kind="ExternalOutput")

    with tile.TileContext(nc) as tc:
        with (
            tc.tile_pool(name="po", bufs=1) as po,
            tc.tile_pool(name="ps", bufs=4, space="PSUM") as pp,
            tc.tile_pool(name="dram", bufs=1, space="DRAM") as dram,
        ):
            # inputs on separate DGE queues so their fixed costs overlap;
            # iotaB (needed later) off the critical path
            Wt = po.tile([12, NROWS], bf16, tag="Wt")
            nc.sync.dma_start(Wt[:], Wt_in[:])
            Xt = po.tile([12, COLS], bf16, tag="Xt")
            nc.scalar.dma_start(Xt[:], Xt_in[:])
            iotaB = po.tile([128, COLS], i16, tag="iotaB")
            nc.scalar.dma_start(iotaB[:], iota_in[0:1, :].to_broadcast((128, COLS)))
            if STORE_MODE == "act":
                # preload the ACT function table during the input DMA wait
                warm = po.tile([1, 1], f32, tag="warm")
                nc.vector.memset(warm[:], 0.0)
                nc.scalar.activation(warm[:], warm[:], AF.Relu, bias=0.0, scale=1.0)

            D = po.tile([128, T * C], i16, tag="D")
            M = [po.tile([128, C], i16, tag=f"M{k}", name=f"M{k}") for k in range(2)]
            M2 = [po.tile([128, C], i16, tag=f"M2{k}", name=f"M2{k}") for k in range(2)]
            Mb = [po.tile([128, C], bf16, tag=f"Mb{k}", name=f"Mb{k}") for k in range(2)]
            l1colf = po.tile([128, T], f32, tag="l1colf")
            l2colf = po.tile([128, T], f32, tag="l2colf")
            l1col = po.tile([128, T], i16, tag="l1col")
            l2col = po.tile([128, T], i16, tag="l2col")
            labelB = po.tile([128, COLS], i16, tag="labelB")
            labelB2 = po.tile([128, COLS], i16, tag="labelB2")
            cnt = po.tile([128, T], f32, tag="cnt")

            l1row = dram.tile([1, NROWS], i16, tag="l1row", name="l1row")
            l2row = dram.tile([1, NROWS], i16, tag="l2row", name="l2row")

            def store(dst, ps, t):
                # GPSIMD cannot read PSUM, so stores are DVE (clamp) or ACT.
                if STORE_MODE == "act":
                    nc.scalar.activation(dst, ps[:], AF.Relu, bias=0.0, scale=1.0)
                else:
                    nc.vector.tensor_scalar(out=dst, in0=ps[:], scalar1=0.0,
                                            scalar2=CLAMP, op0=OP.max, op1=OP.min)

            def tt_engine(t):
                return nc.vector

            def labels_to_bcast(colf, col, row_d, dstB, grp):
                # f32 accum -> i16, then [128, R] -> DRAM row -> bcast tile
                # (group 0 and group 1 chains on different DGE queues)
                q = nc.sync if grp == 0 else nc.scalar
                nc.vector.tensor_copy(col[:, grp * R:(grp + 1) * R],
                                      colf[:, grp * R:(grp + 1) * R])
                q.dma_start(
                    row_d[0:1, grp * R * 128:(grp + 1) * R * 128]
                    .rearrange("o (t p) -> (o p) t", p=128),
                    col[:, grp * R:(grp + 1) * R])
                q.dma_start(
                    dstB[:, grp * C:(grp + 1) * C],
                    row_d[0:1, grp * R * 128:grp * R * 128 + C]
                    .to_broadcast((128, C)))

            # ---- build D + iteration 1 (tile-pipelined) ----
            for t in range(T):
                grp = t // R
                ps = pp.tile([128, C], f32, tag="ps")
                w = Wt[:, t * 128:(t + 1) * 128]
                nc.tensor.matmul(ps[:, 0:512], w, Xt[:, grp * C:grp * C + 512])
                if C > 512:
                    nc.tensor.matmul(ps[:, 512:C], w,
                                     Xt[:, grp * C + 512:grp * C + C])
                dst = D[:, t * C:(t + 1) * C]
                store(dst, ps, t)
                tt_engine(t).tensor_tensor(M[t % 2][:], dst,
                                           iotaB[:, grp * C:(grp + 1) * C], OP.max)
                nc.vector.tensor_scalar(out=M2[t % 2][:], in0=M[t % 2][:],
                                        scalar1=0.0, scalar2=None,
                                        op0=OP.add, op1=OP.min,
                                        accum_out=l1colf[:, t:t + 1])
                if t == R - 1:
                    labels_to_bcast(l1colf, l1col, l1row, labelB, 0)
                elif t == T - 1:
                    labels_to_bcast(l1colf, l1col, l1row, labelB, 1)

            # ---- iteration 2 ----
            for t in range(T):
                grp = t // R
                dst = D[:, t * C:(t + 1) * C]
                tt_engine(t).tensor_tensor(M[t % 2][:], dst,
                                           labelB[:, grp * C:(grp + 1) * C], OP.max)
                nc.vector.tensor_scalar(out=M2[t % 2][:], in0=M[t % 2][:],
                                        scalar1=0.0, scalar2=None,
                                        op0=OP.add, op1=OP.min,
                                        accum_out=l2colf[:, t:t + 1])
                if t == R - 1:
                    labels_to_bcast(l2colf, l2col, l2row, labelB2, 0)
                elif t == T - 1:
                    labels_to_bcast(l2colf, l2col, l2row, labelB2, 1)

            # ---- counts + min-size filter ----
            for t in range(T):
                grp = t // R
                nc.vector.tensor_scalar(out=Mb[t % 2][:],
                                        in0=labelB2[:, grp * C:(grp + 1) * C],
                                        scalar1=l2colf[:, t:t + 1], scalar2=None,
                                        op0=OP.is_equal, op1=OP.add,
                                        accum_out=cnt[:, t:t + 1])
            mask = po.tile([128, T], f32, tag="mask")
            nc.vector.tensor_scalar(out=mask[:], in0=cnt[:], scalar1=2.5,
                                    scalar2=None, op0=OP.is_ge)
            lp1 = po.tile([128, T], f32, tag="lp1")
            nc.vector.tensor_scalar(out=lp1[:], in0=l2colf[:], scalar1=1.0,
                                    scalar2=None, op0=OP.add)
            sel = po.tile([128, T], f32, tag="sel")
            nc.vector.tensor_tensor(sel[:], mask[:], lp1[:], OP.mult)
            outf = po.tile([128, T], f32, tag="outf")
            nc.vector.tensor_scalar(out=outf[:], in0=sel[:], scalar1=-1.0,
                                    scalar2=None, op0=OP.add)
            outi = po.tile([128, T], i32, tag="outi")
            nc.vector.tensor_copy(outi[:], outf[:])
            nc.sync.dma_start(
                out_t[0:1, :].rearrange("o (t p) -> (o p) t", p=128), outi[:])

    nc.compile()
    return nc


def _layout(data):
    """Host-side: stable group sort, padding, bf16 operand prep."""
    import ml_dtypes
    data = np.asarray(data, np.float32)
    N = data.shape[0]
    bid = data[:, 0].astype(np.int64)
    sem = data[:, 4].astype(np.int64)
    xyz = data[:, 1:4].astype(np.int64)
    g = bid * 4 + sem
    order = np.argsort(g, kind="stable")
    sizes = np.bincount(g, minlength=NGROUPS)
    starts = np.concatenate([[0], np.cumsum(sizes)])
    gidx = [order[starts[k]:starts[k + 1]] for k in range(NGROUPS)]

    C = int(sizes.max())
    R = (C + 127) // 128
    T = 2 * R

    # 5D feature rows per point: [x, y, z, W*b, W*s]; pads: [0,0,0,PADB,0]
    def feats(idx, n_slots):
        f = np.zeros((5, n_slots), np.int64)
        k = len(idx)
        f[0:3, :k] = xyz[idx].T
        f[3, :k] = (W * bid[idx]).astype(np.int64)
        f[4, :k] = (W * sem[idx]).astype(np.int64)
        f[3, k:] = int(PADB)
        return f

    in_maps = []
    meta = []
    for c in range(NCORES):
        Wt = np.zeros((12, T * 128), np.float64)
        Xt = np.zeros((12, 2 * C), np.float64)
        for grp in range(2):
            idx = gidx[2 * c + grp]
            fr = feats(idx, R * 128)   # row slots
            fc = feats(idx, C)         # col slots
            qr = (fr * fr).sum(axis=0)
            qc = (fc * fc).sum(axis=0)
            rs, cs = grp * R * 128, grp * C
            Wt[0:5, rs:rs + R * 128] = fr
            Wt[5, rs:rs + R * 128] = qr >> 16
            Wt[6, rs:rs + R * 128] = (qr >> 8) & 255
            Wt[7, rs:rs + R * 128] = qr & 255
            Wt[8:12, rs:rs + R * 128] = 1.0
            Xt[0:5, cs:cs + C] = -2.0 * S * fc
            Xt[5, cs:cs + C] = S * 65536.0
            Xt[6, cs:cs + C] = S * 256.0
            Xt[7, cs:cs + C] = S
            Xt[8, cs:cs + C] = S * 65536.0 * (qc >> 16)
            Xt[9, cs:cs + C] = S * 256.0 * ((qc >> 8) & 255)
            Xt[10, cs:cs + C] = S * (qc & 255)
            Xt[11, cs:cs + C] = -3.0 * S
        Wt_b = Wt.astype(np.float32).astype(ml_dtypes.bfloat16)
        Xt_b = Xt.astype(np.float32).astype(ml_dtypes.bfloat16)
        assert np.array_equal(Wt_b.astype(np.float64), Wt), "Wt not bf16-exact"
        assert np.array_equal(Xt_b.astype(np.float64), Xt), "Xt not bf16-exact"
        iota = np.arange(2 * C, dtype=np.int16).reshape(1, -1)
        in_maps.append({"Wt": Wt_b, "Xt": Xt_b, "iota": iota})
        meta.append((gidx[2 * c], gidx[2 * c + 1]))
    return in_maps, meta, R, C, N


def kernel(data: np.ndarray) -> np.ndarray:
    from concourse.bass_utils import run_bass_kernel_spmd

    in_maps, meta, R, C, N = _layout(data)
    key = ("nc", R, C)
    if key not in _CACHE:
        _CACHE[key] = _build(R, C)
        _CACHE["nc"] = _CACHE[key]
    nc = _CACHE[key]
    res = run_bass_kernel_spmd(nc, in_maps, core_ids=list(range(NCORES)))

    out = np.full(N, -1, np.int32)
    for c in range(NCORES):
        o = np.asarray(res.results[c]["out"]).reshape(-1)
        for grp in range(2):
            idx = meta[c][grp]
            sz = len(idx)
            vals = o[grp * R * 128: grp * R * 128 + sz]
            ok = (vals >= grp * C) & (vals < grp * C + sz)
            out[idx[ok]] = idx[vals[ok] - grp * C]
            out[idx[~ok & (vals >= 0)]] = -2   # unexpected: root outside group
    return out


# revision 9
# speedup vs baseline: 1.3025x; 1.0861x over previous
"""DBSCAN fragmenter (connected components of eps-neighborhood graph) on 8 Trainium2 cores.

Key structural fact: adjacency requires equal batch id AND equal semantic
class, so the graph splits into 16 independent (bid,sem) groups (~512 points
each). Host-side we stably sort points by group and give each core 2 whole
groups (one big + one small, slot sizes uniform across cores); all
propagation is then core-local -- no collectives at all.

Per core (single SPMD program, uniform shapes):
  - slot s (s=0 big, s=1 small): Rs row tiles of 128, Cs columns
    (Cs = max real size of the groups assigned to slot s; pads are far away)
  - D[i,j] = relu(S*(d2(i,j) - 3)) as int16 (HW-saturating at 32767) via one
    K=12 bf16 matmul per tile (exact: coords<=255, q split into 8-bit digits;
    every operand is a small int times a power of two) + one ACT relu store.
  - adjacency (d2<=3, integer d2, eps=1.999) <=> D=0; else D>=8192 > labels.
  - 2 rounds of min-label propagation (component ecc from root <= 2):
    per tile: M = max(D, labels_bcast) [DVE TT, 2x i16 mode], then
    labels_new = free-axis min via tensor_scalar+accum_out [4x mode].
    Label broadcast between rounds via DRAM round-trip + broadcast DMA.
  - counts: per tile tensor_scalar(is_equal)+accum_out(add); out = count>=3 ?
    label : -1.
Labels are core-local column indices; the host maps roots back to original
point indices (stable sort keeps within-group order = original index order).
"""
import sys
sys.path.insert(0, "/opt/trn_rl_repo")
import numpy as np

NCORES = 8
NGROUPS = 16
W = 64.0          # batch/class separation weight ((64*1)^2 = 4096 > 3)
S = 8192.0        # distance scale: S*1 > max local label (< C0+C1 ~ 1100)
PADB = 320.0      # pad-point batch coordinate (W*5): (320-192)^2 from all real
CLAMP = 24576.0   # clamp-mode D cap: > 8191 >= any label; 1231+24576 < 32767
STORE_MODE = "act"     # "act":   ACT relu stores (HW saturates f32->i16)
                       # "clamp": DVE clamped stores (interp-exact, for ctest)

_CACHE = {}


def _build(R0, C0, R1, C1):
    import concourse.bass as bass
    import concourse.bacc as bacc
    import concourse.mybir as mybir
    import concourse.tile as tile

    f32 = mybir.dt.float32
    bf16 = mybir.dt.bfloat16
    i16 = mybir.dt.int16
    i32 = mybir.dt.int32
    OP = mybir.AluOpType
    AF = mybir.ActivationFunctionType

    T = R0 + R1
    COLS = C0 + C1
    NROWS = T * 128
    ROFF = [0, R0]            # slot row-tile offsets
    COFF = [0, C0]            # slot column offsets
    RS = [R0, R1]
    CS = [C0, C1]

    nc = bacc.Bacc("TRN2", target_bir_lowering=False, debug=False,
                   num_devices=NCORES)

    # Wt and Xt fused into one tensor -> one input DMA on the critical path
    WX_in = nc.dram_tensor("WX", [12, NROWS + COLS], bf16, kind="ExternalInput")
    iota_in = nc.dram_tensor("iota", [1, COLS], i16, kind="ExternalInput")
    out_t = nc.dram_tensor("out", [1, NROWS], i32, kind="ExternalOutput")

    with tile.TileContext(nc) as tc:
        with (
            tc.tile_pool(name="po", bufs=1) as po,
            tc.tile_pool(name="ps", bufs=4, space="PSUM") as pp,
            tc.tile_pool(name="dram", bufs=1, space="DRAM") as dram,
        ):
            WX = po.tile([12, NROWS + COLS], bf16, tag="WX")
            nc.sync.dma_start(WX[:], WX_in[:])
            iotaB = po.tile([128, COLS], i16, tag="iotaB")
            nc.scalar.dma_start(iotaB[:], iota_in[0:1, :].to_broadcast((128, COLS)))
            if STORE_MODE == "act":
                # preload the ACT function table during the input DMA wait
                warm = po.tile([1, 1], f32, tag="warm")
                nc.vector.memset(warm[:], 0.0)
                nc.scalar.activation(warm[:], warm[:], AF.Relu, bias=0.0, scale=1.0)

            def Wslice(t):
                return WX[:, t * 128:(t + 1) * 128]

            def Xslice(lo, hi):
                return WX[:, NROWS + lo:NROWS + hi]

            D = po.tile([128, R0 * C0 + R1 * C1], i16, tag="D")

            def Dslice(t):
                if t < R0:
                    return D[:, t * C0:(t + 1) * C0]
                return D[:, R0 * C0 + (t - R0) * C1:R0 * C0 + (t - R0 + 1) * C1]

            M = [po.tile([128, C0], i16, tag=f"M{k}", name=f"M{k}") for k in range(2)]
            M2 = [po.tile([128, C0], i16, tag=f"M2{k}", name=f"M2{k}") for k in range(2)]
            Mb = [po.tile([128, C0], bf16, tag=f"Mb{k}", name=f"Mb{k}") for k in range(2)]
            l1colf = po.tile([128, T], f32, tag="l1colf")
            l2colf = po.tile([128, T], f32, tag="l2colf")
            l1col = po.tile([128, T], i16, tag="l1col")
            l2col = po.tile([128, T], i16, tag="l2col")
            labelB = po.tile([128, COLS], i16, tag="labelB")
            labelB2 = po.tile([128, COLS], i16, tag="labelB2")
            cnt = po.tile([128, T], f32, tag="cnt")

            l1row = dram.tile([1, NROWS], i16, tag="l1row", name="l1row")
            l2row = dram.tile([1, NROWS], i16, tag="l2row", name="l2row")

            def store(dst, ps):
                if STORE_MODE == "act":
                    nc.scalar.activation(dst, ps, AF.Relu, bias=0.0, scale=1.0)
                else:
                    nc.vector.tensor_scalar(out=dst, in0=ps, scalar1=0.0,
                                            scalar2=CLAMP, op0=OP.max, op1=OP.min)

            def labels_to_bcast(colf, col, row_d, dstB, s):
                # f32 accum -> i16, [128, Rs] -> DRAM row -> bcast tile.
                # slot chains ride different DGE queues.
                q = nc.sync if s == 0 else nc.scalar
                r0, r1 = ROFF[s], ROFF[s] + RS[s]
                nc.vector.tensor_copy(col[:, r0:r1], colf[:, r0:r1])
                q.dma_start(
                    row_d[0:1, r0 * 128:r1 * 128]
                    .rearrange("o (t p) -> (o p) t", p=128),
                    col[:, r0:r1])
                q.dma_start(
                    dstB[:, COFF[s]:COFF[s] + CS[s]],
                    row_d[0:1, r0 * 128:r0 * 128 + CS[s]]
                    .to_broadcast((128, CS[s])))

            def tiles():
                for s in range(2):
                    for u in range(RS[s]):
                        yield s, ROFF[s] + u

            # ---- build D + iteration 1 (tile-pipelined) ----
            for s, t in tiles():
                c0, c1 = COFF[s], COFF[s] + CS[s]
                ps = pp.tile([128, CS[s]], f32, tag="ps")
                for lo in range(0, CS[s], 512):
                    hi = min(lo + 512, CS[s])
                    nc.tensor.matmul(ps[:, lo:hi], Wslice(t), Xslice(c0 + lo, c0 + hi))
                dst = Dslice(t)
                store(dst, ps[:])
                nc.vector.tensor_tensor(M[t % 2][:, :CS[s]], dst,
                                        iotaB[:, c0:c1], OP.max)
                nc.vector.tensor_scalar(out=M2[t % 2][:, :CS[s]],
                                        in0=M[t % 2][:, :CS[s]],
                                        scalar1=0.0, scalar2=None,
                                        op0=OP.add, op1=OP.min,
                                        accum_out=l1colf[:, t:t + 1])
                if t == R0 - 1:
                    labels_to_bcast(l1colf, l1col, l1row, labelB, 0)
                elif t == T - 1:
                    labels_to_bcast(l1colf, l1col, l1row, labelB, 1)

            # ---- iteration 2 ----
            for s, t in tiles():
                c0, c1 = COFF[s], COFF[s] + CS[s]
                nc.vector.tensor_tensor(M[t % 2][:, :CS[s]], Dslice(t),
                                        labelB[:, c0:c1], OP.max)
                nc.vector.tensor_scalar(out=M2[t % 2][:, :CS[s]],
                                        in0=M[t % 2][:, :CS[s]],
                                        scalar1=0.0, scalar2=None,
                                        op0=OP.add, op1=OP.min,
                                        accum_out=l2colf[:, t:t + 1])
                if t == R0 - 1:
                    labels_to_bcast(l2colf, l2col, l2row, labelB2, 0)
                elif t == T - 1:
                    labels_to_bcast(l2colf, l2col, l2row, labelB2, 1)

            # ---- counts + min-size filter ----
            for s, t in tiles():
                c0, c1 = COFF[s], COFF[s] + CS[s]
                nc.vector.tensor_scalar(out=Mb[t % 2][:, :CS[s]],
                                        in0=labelB2[:, c0:c1],
                                        scalar1=l2colf[:, t:t + 1], scalar2=None,
                                        op0=OP.is_equal, op1=OP.add,
                                        accum_out=cnt[:, t:t + 1])
            mask = po.tile([128, T], f32, tag="mask")
            nc.vector.tensor_scalar(out=mask[:], in0=cnt[:], scalar1=2.5,
                                    scalar2=None, op0=OP.is_ge)
            lp1 = po.tile([128, T], f32, tag="lp1")
            nc.vector.tensor_scalar(out=lp1[:], in0=l2colf[:], scalar1=1.0,
                                    scalar2=None, op0=OP.add)
            sel = po.tile([128, T], f32, tag="sel")
            nc.vector.tensor_tensor(sel[:], mask[:], lp1[:], OP.mult)
            outf = po.tile([128, T], f32, tag="outf")
            nc.vector.tensor_scalar(out=outf[:], in0=sel[:], scalar1=-1.0,
                                    scalar2=None, op0=OP.add)
            outi = po.tile([128, T], i32, tag="outi")
            nc.vector.tensor_copy(outi[:], outf[:])
            nc.sync.dma_start(
                out_t[0:1, :].rearrange("o (t p) -> (o p) t", p=128), outi[:])

    nc.compile()
    return nc


def _layout(data):
    """Host-side: stable group sort, big/small slot pairing, bf16 operand prep."""
    import ml_dtypes
    data = np.asarray(data, np.float32)
    N = data.shape[0]
    bid = data[:, 0].astype(np.int64)
    sem = data[:, 4].astype(np.int64)
    xyz = data[:, 1:4].astype(np.int64)
    g = bid * 4 + sem
    order = np.argsort(g, kind="stable")
    sizes = np.bincount(g, minlength=NGROUPS)
    starts = np.concatenate([[0], np.cumsum(sizes)])
    gidx = [order[starts[k]:starts[k + 1]] for k in range(NGROUPS)]

    # slot 0 <- the 8 biggest groups, slot 1 <- the 8 smallest;
    # core c gets (big[c], small[NCORES-1-c])
    by_size = sorted(range(NGROUPS), key=lambda k: -sizes[k])
    big, small = by_size[:NCORES], by_size[NCORES:]
    C0 = int(max(sizes[k] for k in big))
    C1 = int(max(sizes[k] for k in small))
    R0 = (C0 + 127) // 128
    R1 = (C1 + 127) // 128
    T = R0 + R1
    RS, CS = [R0, R1], [C0, C1]
    ROFF, COFF = [0, R0], [0, C0]

    def feats(idx, n_slots):
        f = np.zeros((5, n_slots), np.int64)
        k = len(idx)
        f[0:3, :k] = xyz[idx].T
        f[3, :k] = (W * bid[idx]).astype(np.int64)
        f[4, :k] = (W * sem[idx]).astype(np.int64)
        f[3, k:] = int(PADB)
        return f

    in_maps = []
    meta = []
    for c in range(NCORES):
        groups = (gidx[big[c]], gidx[small[NCORES - 1 - c]])
        Wt = np.zeros((12, T * 128), np.float64)
        Xt = np.zeros((12, C0 + C1), np.float64)
        for s in range(2):
            idx = groups[s]
            fr = feats(idx, RS[s] * 128)
            fc = feats(idx, CS[s])
            qr = (fr * fr).sum(axis=0)
            qc = (fc * fc).sum(axis=0)
            rs, cs = ROFF[s] * 128, COFF[s]
            re, ce = rs + RS[s] * 128, cs + CS[s]
            Wt[0:5, rs:re] = fr
            Wt[5, rs:re] = qr >> 16
            Wt[6, rs:re] = (qr >> 8) & 255
            Wt[7, rs:re] = qr & 255
            Wt[8:12, rs:re] = 1.0
            Xt[0:5, cs:ce] = -2.0 * S * fc
            Xt[5, cs:ce] = S * 65536.0
            Xt[6, cs:ce] = S * 256.0
            Xt[7, cs:ce] = S
            Xt[8, cs:ce] = S * 65536.0 * (qc >> 16)
            Xt[9, cs:ce] = S * 256.0 * ((qc >> 8) & 255)
            Xt[10, cs:ce] = S * (qc & 255)
            Xt[11, cs:ce] = -3.0 * S
        WX = np.concatenate([Wt, Xt], axis=1)
        WX_b = WX.astype(np.float32).astype(ml_dtypes.bfloat16)
        assert np.array_equal(WX_b.astype(np.float64), WX), "WX not bf16-exact"
        iota = np.arange(C0 + C1, dtype=np.int16).reshape(1, -1)
        in_maps.append({"WX": WX_b, "iota": iota})
        meta.append(groups)
    return in_maps, meta, (R0, C0, R1, C1), N


def kernel(data: np.ndarray) -> np.ndarray:
    from concourse.bass_utils import run_bass_kernel_spmd

    in_maps, meta, dims, N = _layout(data)
    R0, C0, R1, C1 = dims
    key = ("nc",) + dims
    if key not in _CACHE:
        _CACHE[key] = _build(*dims)
        _CACHE["nc"] = _CACHE[key]
    nc = _CACHE[key]
    res = run_bass_kernel_spmd(nc, in_maps, core_ids=list(range(NCORES)))

    ROFF, COFF = [0, R0], [0, C0]
    out = np.full(N, -1, np.int32)
    for c in range(NCORES):
        o = np.asarray(res.results[c]["out"]).reshape(-1)
        for s in range(2):
            idx = meta[c][s]
            sz = len(idx)
            vals = o[ROFF[s] * 128: ROFF[s] * 128 + sz]
            ok = (vals >= COFF[s]) & (vals < COFF[s] + sz)
            out[idx[ok]] = idx[vals[ok] - COFF[s]]
            out[idx[~ok & (vals >= 0)]] = -2   # unexpected: root outside group
    return out


# revision 10
# speedup vs baseline: 1.6930x; 1.2998x over previous
"""DBSCAN fragmenter (connected components of eps-neighborhood graph) on 8 Trainium2 cores.

Key structural fact: adjacency requires equal batch id AND equal semantic
class, so the graph splits into 16 independent (bid,sem) groups (~512 points
each). Host-side we stably sort points by group and give each core 2 whole
groups (one big + one small, slot sizes uniform across cores); all
propagation is then core-local -- no collectives at all.

Per core (single SPMD program, uniform shapes):
  - slot s (s=0 big, s=1 small): Rs row tiles of 128, Cs columns
    (Cs = max real size of the groups assigned to slot s; pads are far away)
  - D[i,j] = relu(S*(d2(i,j) - 3)) as int16 (HW-saturating at 32767) via one
    K=12 bf16 matmul per tile (exact: coords<=255, q split into 8-bit digits;
    every operand is a small int times a power of two) + one ACT relu store.
  - adjacency (d2<=3, integer d2, eps=1.999) <=> D=0; else D>=8192 > labels.
  - 2 rounds of min-label propagation (component ecc from root <= 2):
    per tile: M = max(D, labels_bcast) [DVE TT, 2x i16 mode], then
    labels_new = free-axis min via tensor_scalar+accum_out [4x mode].
    Label broadcast between rounds via DRAM round-trip + broadcast DMA.
  - counts: per tile tensor_scalar(is_equal)+accum_out(add); out = count>=3 ?
    label : -1.
Labels are core-local column indices; the host maps roots back to original
point indices (stable sort keeps within-group order = original index order).
"""
import sys
sys.path.insert(0, "/opt/trn_rl_repo")
import numpy as np

NCORES = 8
NGROUPS = 16
W = 64.0          # batch/class separation weight ((64*1)^2 = 4096 > 3)
S = 8192.0        # distance scale: S*1 > max local label (< C0+C1 ~ 1100)
PADB = 320.0      # pad-point batch coordinate (W*5): (320-192)^2 from all real
CLAMP = 24576.0   # clamp-mode D cap: > 8191 >= any label; 1231+24576 < 32767
STORE_MODE = "act"     # "act":   ACT relu stores (HW saturates f32->i16)
                       # "clamp": DVE clamped stores (interp-exact, for ctest)

_CACHE = {}


def _build(R0, C0, R1, C1):
    import concourse.bass as bass
    import concourse.bacc as bacc
    import concourse.mybir as mybir
    import concourse.tile as tile

    f32 = mybir.dt.float32
    bf16 = mybir.dt.bfloat16
    f16 = mybir.dt.float16
    i16 = mybir.dt.int16
    i32 = mybir.dt.int32
    OP = mybir.AluOpType
    AF = mybir.ActivationFunctionType

    T = R0 + R1
    COLS = C0 + C1
    NROWS = T * 128
    ROFF = [0, R0]            # slot row-tile offsets
    COFF = [0, C0]            # slot column offsets
    RS = [R0, R1]
    CS = [C0, C1]

    nc = bacc.Bacc("TRN2", target_bir_lowering=False, debug=False,
                   num_devices=NCORES)

    # Wt and Xt fused into one tensor -> one input DMA on the critical path
    WX_in = nc.dram_tensor("WX", [12, NROWS + COLS], bf16, kind="ExternalInput")
    iota_in = nc.dram_tensor("iota", [1, COLS], i16, kind="ExternalInput")
    ident_in = nc.dram_tensor("ident", [128, 128], f32, kind="ExternalInput")
    sel_in = nc.dram_tensor("sel", [R0, R0 * 128], f16, kind="ExternalInput")
    out_t = nc.dram_tensor("out", [1, NROWS], i32, kind="ExternalOutput")

    with tile.TileContext(nc) as tc:
        with (
            tc.tile_pool(name="po", bufs=1) as po,
            tc.tile_pool(name="ps", bufs=2, space="PSUM") as pp,
            tc.tile_pool(name="psT", bufs=1, space="PSUM") as ppT,
            tc.tile_pool(name="psB", bufs=1, space="PSUM") as ppB,
        ):
            WX = po.tile([12, NROWS + COLS], bf16, tag="WX")
            nc.sync.dma_start(WX[:], WX_in[:])
            iotaB = po.tile([128, COLS], i16, tag="iotaB")
            nc.scalar.dma_start(iotaB[:], iota_in[0:1, :].to_broadcast((128, COLS)))
            ident = po.tile([128, 128], f32, tag="ident")
            nc.scalar.dma_start(ident[:], ident_in[:])
            sel = po.tile([R0, R0 * 128], f16, tag="sel")
            nc.scalar.dma_start(sel[:], sel_in[:])
            if STORE_MODE == "act":
                # preload the ACT function table during the input DMA wait
                warm = po.tile([1, 1], f32, tag="warm")
                nc.vector.memset(warm[:], 0.0)
                nc.scalar.activation(warm[:], warm[:], AF.Relu, bias=0.0, scale=1.0)

            def Wslice(t):
                return WX[:, t * 128:(t + 1) * 128]

            def Xslice(lo, hi):
                return WX[:, NROWS + lo:NROWS + hi]

            D = po.tile([128, R0 * C0 + R1 * C1], i16, tag="D")

            def Dslice(t):
                if t < R0:
                    return D[:, t * C0:(t + 1) * C0]
                return D[:, R0 * C0 + (t - R0) * C1:R0 * C0 + (t - R0 + 1) * C1]

            M = [po.tile([128, C0], i16, tag=f"M{k}", name=f"M{k}") for k in range(2)]
            M2 = [po.tile([128, C0], i16, tag=f"M2{k}", name=f"M2{k}") for k in range(2)]
            Mb = [po.tile([128, C0], bf16, tag=f"Mb{k}", name=f"Mb{k}") for k in range(2)]
            l1colf = po.tile([128, T], f32, tag="l1colf")
            l2colf = po.tile([128, T], f32, tag="l2colf")
            rowT = [po.tile([R0, 128], f16, tag=f"rowT{k}", name=f"rowT{k}")
                    for k in range(2)]
            labelB = po.tile([128, COLS], i16, tag="labelB")
            labelB2 = po.tile([128, COLS], i16, tag="labelB2")
            cnt = po.tile([128, T], f32, tag="cnt")

            def store(dst, ps):
                if STORE_MODE == "act":
                    nc.scalar.activation(dst, ps, AF.Relu, bias=0.0, scale=1.0)
                else:
                    nc.vector.tensor_scalar(out=dst, in0=ps, scalar1=0.0,
                                            scalar2=CLAMP, op0=OP.max, op1=OP.min)

            def labels_to_bcast(colf, dstB, s):
                # PE transpose + one-hot-sel matmuls broadcast the slot's
                # labels along partitions (no DRAM hop, engine-only sems):
                # psT[u,q] = colf[q, ROFF+u]; psB[p, u*128+q] = psT[u, q].
                r0, rn = ROFF[s], RS[s]
                psT = ppT.tile([R0, 128], f32, tag="psT")
                nc.tensor.transpose(psT[0:rn, :], colf[:, r0:r0 + rn], ident[:])
                rT = rowT[s]
                nc.vector.tensor_copy(rT[0:rn, :], psT[0:rn, :])
                psB = ppB.tile([128, R0 * 128], f32, tag="psB")
                for u in range(rn):
                    nc.tensor.matmul(psB[:, u * 128:(u + 1) * 128],
                                     sel[0:rn, u * 128:u * 128 + 128],
                                     rT[0:rn, :])
                nc.scalar.activation(dstB[:, COFF[s]:COFF[s] + CS[s]],
                                     psB[:, 0:CS[s]], AF.Copy, bias=0.0,
                                     scale=1.0)

            def tiles():
                for s in range(2):
                    for u in range(RS[s]):
                        yield s, ROFF[s] + u

            # ---- build D + iteration 1 (tile-pipelined) ----
            for s, t in tiles():
                c0, c1 = COFF[s], COFF[s] + CS[s]
                ps = pp.tile([128, CS[s]], f32, tag="ps")
                for lo in range(0, CS[s], 512):
                    hi = min(lo + 512, CS[s])
                    nc.tensor.matmul(ps[:, lo:hi], Wslice(t), Xslice(c0 + lo, c0 + hi))
                dst = Dslice(t)
                store(dst, ps[:])
                nc.vector.tensor_tensor(M[t % 2][:, :CS[s]], dst,
                                        iotaB[:, c0:c1], OP.max)
                nc.vector.tensor_scalar(out=M2[t % 2][:, :CS[s]],
                                        in0=M[t % 2][:, :CS[s]],
                                        scalar1=0.0, scalar2=None,
                                        op0=OP.add, op1=OP.min,
                                        accum_out=l1colf[:, t:t + 1])
                if t == R0 - 1:
                    labels_to_bcast(l1colf, labelB, 0)
                elif t == T - 1:
                    labels_to_bcast(l1colf, labelB, 1)

            # ---- iteration 2 ----
            for s, t in tiles():
                c0, c1 = COFF[s], COFF[s] + CS[s]
                nc.vector.tensor_tensor(M[t % 2][:, :CS[s]], Dslice(t),
                                        labelB[:, c0:c1], OP.max)
                nc.vector.tensor_scalar(out=M2[t % 2][:, :CS[s]],
                                        in0=M[t % 2][:, :CS[s]],
                                        scalar1=0.0, scalar2=None,
                                        op0=OP.add, op1=OP.min,
                                        accum_out=l2colf[:, t:t + 1])
                if t == R0 - 1:
                    labels_to_bcast(l2colf, labelB2, 0)
                elif t == T - 1:
                    labels_to_bcast(l2colf, labelB2, 1)

            # ---- counts + min-size filter ----
            for s, t in tiles():
                c0, c1 = COFF[s], COFF[s] + CS[s]
                nc.vector.tensor_scalar(out=Mb[t % 2][:, :CS[s]],
                                        in0=labelB2[:, c0:c1],
                                        scalar1=l2colf[:, t:t + 1], scalar2=None,
                                        op0=OP.is_equal, op1=OP.add,
                                        accum_out=cnt[:, t:t + 1])
            mask = po.tile([128, T], f32, tag="mask")
            nc.vector.tensor_scalar(out=mask[:], in0=cnt[:], scalar1=2.5,
                                    scalar2=None, op0=OP.is_ge)
            lp1 = po.tile([128, T], f32, tag="lp1")
            nc.vector.tensor_scalar(out=lp1[:], in0=l2colf[:], scalar1=1.0,
                                    scalar2=None, op0=OP.add)
            sel = po.tile([128, T], f32, tag="sel")
            nc.vector.tensor_tensor(sel[:], mask[:], lp1[:], OP.mult)
            outf = po.tile([128, T], f32, tag="outf")
            nc.vector.tensor_scalar(out=outf[:], in0=sel[:], scalar1=-1.0,
                                    scalar2=None, op0=OP.add)
            outi = po.tile([128, T], i32, tag="outi")
            nc.vector.tensor_copy(outi[:], outf[:])
            nc.sync.dma_start(
                out_t[0:1, :].rearrange("o (t p) -> (o p) t", p=128), outi[:])

    nc.compile()
    return nc


def _layout(data):
    """Host-side: stable group sort, big/small slot pairing, bf16 operand prep."""
    import ml_dtypes
    data = np.asarray(data, np.float32)
    N = data.shape[0]
    bid = data[:, 0].astype(np.int64)
    sem = data[:, 4].astype(np.int64)
    xyz = data[:, 1:4].astype(np.int64)
    g = bid * 4 + sem
    order = np.argsort(g, kind="stable")
    sizes = np.bincount(g, minlength=NGROUPS)
    starts = np.concatenate([[0], np.cumsum(sizes)])
    gidx = [order[starts[k]:starts[k + 1]] for k in range(NGROUPS)]

    # slot 0 <- the 8 biggest groups, slot 1 <- the 8 smallest;
    # core c gets (big[c], small[NCORES-1-c])
    by_size = sorted(range(NGROUPS), key=lambda k: -sizes[k])
    big, small = by_size[:NCORES], by_size[NCORES:]
    C0 = int(max(sizes[k] for k in big))
    C1 = int(max(sizes[k] for k in small))
    R0 = (C0 + 127) // 128
    R1 = (C1 + 127) // 128
    T = R0 + R1
    RS, CS = [R0, R1], [C0, C1]
    ROFF, COFF = [0, R0], [0, C0]

    def feats(idx, n_slots):
        f = np.zeros((5, n_slots), np.int64)
        k = len(idx)
        f[0:3, :k] = xyz[idx].T
        f[3, :k] = (W * bid[idx]).astype(np.int64)
        f[4, :k] = (W * sem[idx]).astype(np.int64)
        f[3, k:] = int(PADB)
        return f

    in_maps = []
    meta = []
    for c in range(NCORES):
        groups = (gidx[big[c]], gidx[small[NCORES - 1 - c]])
        Wt = np.zeros((12, T * 128), np.float64)
        Xt = np.zeros((12, C0 + C1), np.float64)
        for s in range(2):
            idx = groups[s]
            fr = feats(idx, RS[s] * 128)
            fc = feats(idx, CS[s])
            qr = (fr * fr).sum(axis=0)
            qc = (fc * fc).sum(axis=0)
            rs, cs = ROFF[s] * 128, COFF[s]
            re, ce = rs + RS[s] * 128, cs + CS[s]
            Wt[0:5, rs:re] = fr
            Wt[5, rs:re] = qr >> 16
            Wt[6, rs:re] = (qr >> 8) & 255
            Wt[7, rs:re] = qr & 255
            Wt[8:12, rs:re] = 1.0
            Xt[0:5, cs:ce] = -2.0 * S * fc
            Xt[5, cs:ce] = S * 65536.0
            Xt[6, cs:ce] = S * 256.0
            Xt[7, cs:ce] = S
            Xt[8, cs:ce] = S * 65536.0 * (qc >> 16)
            Xt[9, cs:ce] = S * 256.0 * ((qc >> 8) & 255)
            Xt[10, cs:ce] = S * (qc & 255)
            Xt[11, cs:ce] = -3.0 * S
        WX = np.concatenate([Wt, Xt], axis=1)
        WX_b = WX.astype(np.float32).astype(ml_dtypes.bfloat16)
        assert np.array_equal(WX_b.astype(np.float64), WX), "WX not bf16-exact"
        iota = np.arange(C0 + C1, dtype=np.int16).reshape(1, -1)
        ident = np.eye(128, dtype=np.float32)
        sel = np.zeros((R0, R0 * 128), np.float16)
        for u in range(R0):
            sel[u, u * 128:(u + 1) * 128] = 1.0
        in_maps.append({"WX": WX_b, "iota": iota, "ident": ident, "sel": sel})
        meta.append(groups)
    return in_maps, meta, (R0, C0, R1, C1), N


def kernel(data: np.ndarray) -> np.ndarray:
    from concourse.bass_utils import run_bass_kernel_spmd

    in_maps, meta, dims, N = _layout(data)
    R0, C0, R1, C1 = dims
    key = ("nc",) + dims
    if key not in _CACHE:
        _CACHE[key] = _build(*dims)
        _CACHE["nc"] = _CACHE[key]
    nc = _CACHE[key]
    res = run_bass_kernel_spmd(nc, in_maps, core_ids=list(range(NCORES)))

    ROFF, COFF = [0, R0], [0, C0]
    out = np.full(N, -1, np.int32)
    for c in range(NCORES):
        o = np.asarray(res.results[c]["out"]).reshape(-1)
        for s in range(2):
            idx = meta[c][s]
            sz = len(idx)
            vals = o[ROFF[s] * 128: ROFF[s] * 128 + sz]
            ok = (vals >= COFF[s]) & (vals < COFF[s] + sz)
            out[idx[ok]] = idx[vals[ok] - COFF[s]]
            out[idx[~ok & (vals >= 0)]] = -2   # unexpected: root outside group
    return out
